# revision 5
# baseline (speedup 1.0000x reference)
"""Multi-head attention (no softmax) on 8 TRN2 NeuronCores.

Problem: x[2,2048,1024], per-head Wq/Wk/Wv[16,64,1024] + biases.
    q = einsum('bsd,hed->bhse', x, Wq) + bq   (same for k, v)
    out = ((q @ k^T) * E^-0.5) @ v, heads concatenated on feature dim.

Key algebraic fact: there is NO softmax, so
    (q k^T * norm) v = q @ (norm * (k^T v))
which collapses the O(S^2) attention into a 64x64 (per head) matmul.

Sharding: 2D tensor-parallel over (batch, head-quad): core c owns batch
c//4 and heads 4*(c%4) .. 4*(c%4)+3.  Each core reads only its batch's
half of x (8.4MB), and processes its 4 heads as two packed head-pairs
(feature groups g=0,1 of 128).

Per core:
  phase 1: project QT/KT/VT[g] = W[g] @ x_b^T in [feat(128), seq(2048)]
           layout (fp32r matmuls, N=512 moving dim -> full PE speed,
           d-major inner loop so the stationary weights are reused by 4
           consecutive matmuls).  norm is folded into Wq/bq on the host;
           biases are per-partition adds fused into the PSUM->SBUF copies.
  phase 2: per g: PE-transpose K/V tiles to [seq, feat] and accumulate
           M_g = K^T V [128,128] in PSUM over 16 seq-chunks; copy the two
           diagonal 64x64 head blocks into a zeroed SBUF tile (cross-head
           blocks of M are garbage and must be dropped).
  phase 3: outT[g][:, s-chunk] = M_g(blockdiag).T @ QT[g][:, s-chunk].
Host gathers: out[c//4, s, (c%4)*256+g*128 +:128] = outT_c[g][:, s].T
"""

import numpy as np

import concourse.bacc as bacc
import concourse.tile as tile
import concourse.mybir as mybir
from concourse import bass2jax

B, S, D, H = 2, 2048, 1024, 16
E = 64          # head dim
NCORES = 8
NB = NCORES // B            # cores per batch (4)
HL = H // NB                # heads per core (4)
NG = 2                      # feature groups per core (head pairs)
EP = HL * E // NG           # packed feature dim per group (128)
P = 128                     # partitions
DC = D // P                 # d chunks (8)
SC = 512                    # seq chunk for N=512 matmuls
NSC = S // SC               # 4 seq chunks
HW = 1024                   # phase-1 psum half-width (2 banks)
NHW = S // HW               # 2 halves
TC = S // P                 # 16 transpose chunks per group
NORM = float(E) ** -0.5

F32 = mybir.dt.float32
F32R = mybir.dt.float32r

_compiled = None


def _build():
    nc = bacc.Bacc("TRN2", target_bir_lowering=False, debug=False)

    x_d = nc.dram_tensor("x", [DC, NSC, P, SC], F32R, kind="ExternalInput").ap()
    w_d = {}
    for g in range(NG):
        for wn in ("wq", "wk", "wv"):
            w_d[wn, g] = nc.dram_tensor(
                f"{wn}{g}", [P, DC, P], F32R, kind="ExternalInput").ap()
    b_d = {}
    for g in range(NG):
        for bn in ("bq", "bk", "bv"):
            b_d[bn, g] = nc.dram_tensor(
                f"{bn}{g}", [P, 1], F32, kind="ExternalInput").ap()
    id_d = nc.dram_tensor("ident", [P, P], F32R, kind="ExternalInput").ap()
    out_d = nc.dram_tensor("outT", [NG, P, S], F32, kind="ExternalOutput").ap()

    with tile.TileContext(nc) as tc:
        with (
            tc.tile_pool(name="consts", bufs=1) as consts,
            tc.tile_pool(name="xs", bufs=32) as xs_pool,
            tc.tile_pool(name="qkv", bufs=1) as qkv_pool,
            tc.tile_pool(name="kv", bufs=4) as kv_pool,
            tc.tile_pool(name="mt", bufs=1) as mt_pool,
            tc.tile_pool(name="ot", bufs=3) as ot_pool,
            tc.tile_pool(name="pproj", bufs=2, space="PSUM") as pproj,
            tc.tile_pool(name="ptr", bufs=2, space="PSUM") as ptr,
            tc.tile_pool(name="pm", bufs=2, space="PSUM") as pm,
        ):
            # ---- x stream DMAs for the first half, then weights, then rest.
            # Issue order matters: the DMA queue drains in order, and the
            # first matmul only needs wq0 + x[0,0].
            xs = {}

            def load_x(i, j):
                t = xs_pool.tile([P, SC], F32R, tag="xs", name=f"x_{i}_{j}")
                nc.sync.dma_start(t[:], x_d[i, j])
                xs[i, j] = t

            w_tiles, b_tiles = {}, {}

            def load_w(wn, g):
                wt = consts.tile([P, DC, P], F32R, tag=f"{wn}{g}", name=f"{wn}{g}_t")
                nc.sync.dma_start(wt[:], w_d[wn, g][:])
                w_tiles[wn, g] = wt

            load_w("wq", 0)
            for i in range(DC):
                load_x(i, 0)
            load_w("wk", 0)
            load_w("wv", 0)
            for j in range(1, NSC):
                for i in range(DC):
                    load_x(i, j)
            load_w("wq", 1)
            load_w("wk", 1)
            load_w("wv", 1)
            for g in range(NG):
                for bn in ("bq", "bk", "bv"):
                    bt = consts.tile([P, 1], F32, tag=f"{bn}{g}", name=f"{bn}{g}_t")
                    nc.sync.dma_start(bt[:], b_d[bn, g][:])
                    b_tiles[bn, g] = bt
            ident = consts.tile([P, P], F32R, tag="ident")
            nc.sync.dma_start(ident[:], id_d[:])

            # ---- phase 1: QT/KT/VT projections, d-major for weight reuse
            big = {}
            for g in range(NG):
                for tn in ("q", "k", "v"):
                    big[tn, g] = qkv_pool.tile([P, S], F32R, tag=f"{tn}t{g}",
                                               name=f"{tn}t{g}")
            for g in range(NG):
                for tn, wn, bn in (("q", "wq", "bq"), ("k", "wk", "bk"),
                                   ("v", "wv", "bv")):
                    for h in range(NHW):
                        ps = pproj.tile([P, HW], F32, tag="proj",
                                        name=f"ps_{tn}{g}_{h}")
                        for jj in range(HW // SC):
                            j = h * (HW // SC) + jj
                            for i in range(DC):
                                nc.tensor.matmul(
                                    ps[:, jj * SC:(jj + 1) * SC],
                                    w_tiles[wn, g][:, i, :], xs[i, j][:],
                                    start=(i == 0), stop=(i == DC - 1),
                                )
                        sl = big[tn, g][:, h * HW:(h + 1) * HW]
                        if tn == "v":
                            nc.scalar.activation(
                                sl, ps[:], mybir.ActivationFunctionType.Identity,
                                bias=b_tiles[bn, g][:])
                        else:
                            nc.vector.tensor_scalar_add(sl, ps[:], b_tiles[bn, g][:])

            # ---- phase 2: M_g = K^T V via PE transposes
            m_tiles = {}
            for g in range(NG):
                mps = pm.tile([P, P], F32, tag="m", name=f"mps_{g}")
                for t in range(TC):
                    sl = slice(t * P, (t + 1) * P)
                    ktp = ptr.tile([P, P], F32R, tag="tr", name=f"ktp_{g}_{t}")
                    nc.tensor.transpose(ktp[:], big["k", g][:, sl], ident[:])
                    k_sb = kv_pool.tile([P, P], F32R, tag="k_sb", name=f"k_sb_{g}_{t}")
                    nc.scalar.copy(k_sb[:], ktp[:])
                    vtp = ptr.tile([P, P], F32R, tag="tr", name=f"vtp_{g}_{t}")
                    nc.tensor.transpose(vtp[:], big["v", g][:, sl], ident[:])
                    v_sb = kv_pool.tile([P, P], F32R, tag="v_sb", name=f"v_sb_{g}_{t}")
                    nc.vector.tensor_copy(v_sb[:], vtp[:])
                    nc.tensor.matmul(mps[:], k_sb[:], v_sb[:],
                                     start=(t == 0), stop=(t == TC - 1))
                mt = mt_pool.tile([P, P], F32R, tag=f"mt{g}", name=f"mt_{g}")
                # zero-fill without InstMemset (walrus rejects f32r memset)
                nc.vector.tensor_scalar_mul(mt[:], ident[:], 0.0)
                nc.vector.tensor_copy(mt[0:E, 0:E], mps[0:E, 0:E])
                nc.vector.tensor_copy(mt[E:P, E:P], mps[E:P, E:P])
                m_tiles[g] = mt

            # ---- phase 3: outT[g] = M_g.T @ QT[g]
            for g in range(NG):
                for j in range(NSC):
                    sl = slice(j * SC, (j + 1) * SC)
                    ps = pproj.tile([P, SC], F32, tag="proj", name=f"ops_{g}_{j}")
                    nc.tensor.matmul(ps[:], m_tiles[g][:], big["q", g][:, sl],
                                     start=True, stop=True)
                    ot = ot_pool.tile([P, SC], F32, tag="ot", name=f"ot_{g}_{j}")
                    nc.vector.tensor_copy(ot[:], ps[:])
                    nc.sync.dma_start(out_d[g, :, sl], ot[:])

    nc.compile()
    return nc


def _prep_inputs(x, Wq, Wk, Wv, bq, bk, bv):
    """Host-side shard + layout prep. Returns per-core input maps."""
    x_tiles_b = []
    for b in range(B):
        xf = np.ascontiguousarray(x[b].T)                   # [D, S]
        x_tiles_b.append(np.ascontiguousarray(
            xf.reshape(DC, P, NSC, SC).transpose(0, 2, 1, 3)))

    def wlayout(w):                                         # [P, D] -> [P, DC, P]
        return np.ascontiguousarray(w.T.reshape(DC, P, P).transpose(1, 0, 2))

    in_maps = []
    for c in range(NCORES):
        b = c // NB
        q0 = HL * (c % NB)                                  # first head of core
        m = {"x": x_tiles_b[b], "ident": np.eye(P, dtype=np.float32)}
        for g in range(NG):
            hs = slice(q0 + 2 * g, q0 + 2 * g + 2)
            m[f"wq{g}"] = wlayout((Wq[hs].reshape(P, D) * NORM).astype(np.float32))
            m[f"wk{g}"] = wlayout(Wk[hs].reshape(P, D).astype(np.float32))
            m[f"wv{g}"] = wlayout(Wv[hs].reshape(P, D).astype(np.float32))
            m[f"bq{g}"] = (bq[hs].reshape(P, 1) * NORM).astype(np.float32)
            m[f"bk{g}"] = bk[hs].reshape(P, 1).astype(np.float32)
            m[f"bv{g}"] = bv[hs].reshape(P, 1).astype(np.float32)
        in_maps.append(m)
    return in_maps


def _gather(results):
    out = np.empty((B, S, D), dtype=np.float32)
    for c in range(NCORES):
        b = c // NB
        oc = results[c]["outT"]                             # [NG, P, S]
        for g in range(NG):
            f0 = (c % NB) * (HL * E) + g * P
            out[b, :, f0:f0 + P] = oc[g].T
    return out


def get_compiled():
    global _compiled
    if _compiled is None:
        _compiled = _build()
    return _compiled


def run(in_maps):
    nc = get_compiled()
    return bass2jax.run_bass_via_pjrt(nc, in_maps, n_cores=NCORES)


def kernel(x, Wq, Wk, Wv, bq, bk, bv):
    in_maps = _prep_inputs(
        np.asarray(x, np.float32), np.asarray(Wq, np.float32),
        np.asarray(Wk, np.float32), np.asarray(Wv, np.float32),
        np.asarray(bq, np.float32), np.asarray(bk, np.float32),
        np.asarray(bv, np.float32),
    )
    return _gather(run(in_maps))


# revision 8
# speedup vs baseline: 1.2377x; 1.2377x over previous
"""Multi-head attention (no softmax) on 8 TRN2 NeuronCores.

Problem: x[2,2048,1024], per-head Wq/Wk/Wv[16,64,1024] + biases.
    q = einsum('bsd,hed->bhse', x, Wq) + bq   (same for k, v)
    out = ((q @ k^T) * E^-0.5) @ v, heads concatenated on feature dim.

Key algebraic fact: there is NO softmax, so
    (q k^T * norm) v = q @ (norm * (k^T v))
which collapses the O(S^2) attention into a 64x64 (per head) matmul.

Sharding: 2D tensor-parallel over (batch, head-quad): core c owns batch
c//4 and heads 4*(c%4) .. 4*(c%4)+3.  Each core reads only its batch's
half of x (8.4MB), and processes its 4 heads as two packed head-pairs
(feature groups g=0,1 of 128).

Per core:
  phase 1: project QT/KT/VT[g] = W[g] @ x_b^T in [feat(128), seq(2048)]
           layout (fp32r matmuls, N=512 moving dim -> full PE speed,
           d-major inner loop so the stationary weights are reused by 4
           consecutive matmuls).  norm is folded into Wq/bq on the host;
           biases are per-partition adds fused into the PSUM->SBUF copies.
  phase 2: per g: PE-transpose K/V tiles to [seq, feat] and accumulate
           M_g = K^T V [128,128] in PSUM over 16 seq-chunks; copy the two
           diagonal 64x64 head blocks into a zeroed SBUF tile (cross-head
           blocks of M are garbage and must be dropped).
  phase 3: outT[g][:, s-chunk] = M_g(blockdiag).T @ QT[g][:, s-chunk].
Host gathers: out[c//4, s, (c%4)*256+g*128 +:128] = outT_c[g][:, s].T
"""

import numpy as np

import concourse.bacc as bacc
import concourse.tile as tile
import concourse.mybir as mybir
from concourse import bass2jax

B, S, D, H = 2, 2048, 1024, 16
E = 64          # head dim
NCORES = 8
NB = NCORES // B            # cores per batch (4)
HL = H // NB                # heads per core (4)
NG = 2                      # feature groups per core (head pairs)
EP = HL * E // NG           # packed feature dim per group (128)
P = 128                     # partitions
DC = D // P                 # d chunks (8)
SC = 512                    # seq chunk for N=512 matmuls
NSC = S // SC               # 4 seq chunks
HW = 1024                   # phase-1 psum half-width (2 banks)
NHW = S // HW               # 2 halves
TC = S // P                 # 16 transpose chunks per group
NORM = float(E) ** -0.5

F32 = mybir.dt.float32
F32R = mybir.dt.float32r

_compiled = None


def _build():
    nc = bacc.Bacc("TRN2", target_bir_lowering=False, debug=False)

    x_d = nc.dram_tensor("x", [DC, NSC, P, SC], F32R, kind="ExternalInput").ap()
    w_d = {}
    for g in range(NG):
        for wn in ("wq", "wk", "wv"):
            w_d[wn, g] = nc.dram_tensor(
                f"{wn}{g}", [P, DC, P], F32R, kind="ExternalInput").ap()
    b_d = {}
    for g in range(NG):
        for bn in ("bq", "bk", "bv"):
            b_d[bn, g] = nc.dram_tensor(
                f"{bn}{g}", [P, 1], F32, kind="ExternalInput").ap()
    id_d = nc.dram_tensor("ident", [P, P], F32R, kind="ExternalInput").ap()
    out_d = nc.dram_tensor("outT", [NG, P, S], F32, kind="ExternalOutput").ap()

    with tile.TileContext(nc) as tc:
        with (
            tc.tile_pool(name="consts", bufs=1) as consts,
            tc.tile_pool(name="xs", bufs=32) as xs_pool,
            tc.tile_pool(name="qkv", bufs=1) as qkv_pool,
            tc.tile_pool(name="kv", bufs=4) as kv_pool,
            tc.tile_pool(name="mt", bufs=1) as mt_pool,
            tc.tile_pool(name="ot", bufs=3) as ot_pool,
            tc.tile_pool(name="pproj", bufs=4, space="PSUM") as pproj,
            tc.tile_pool(name="ptr", bufs=2, space="PSUM") as ptr,
            tc.tile_pool(name="pm", bufs=2, space="PSUM") as pm,
        ):
            # ---- x stream DMAs for the first half, then weights, then rest.
            # Issue order matters: the DMA queue drains in order, and the
            # first matmul only needs wq0 + x[0,0].
            xs = {}

            def load_x(i, j):
                t = xs_pool.tile([P, SC], F32R, tag="xs", name=f"x_{i}_{j}")
                nc.sync.dma_start(t[:], x_d[i, j])
                xs[i, j] = t

            w_tiles, b_tiles = {}, {}

            def load_w(wn, g):
                wt = consts.tile([P, DC, P], F32R, tag=f"{wn}{g}", name=f"{wn}{g}_t")
                nc.sync.dma_start(wt[:], w_d[wn, g][:])
                w_tiles[wn, g] = wt

            load_w("wq", 0)
            for i in range(DC):
                load_x(i, 0)
            load_w("wk", 0)
            load_w("wv", 0)
            for g in range(NG):
                for bn in ("bq", "bk", "bv"):
                    bt = consts.tile([P, 1], F32, tag=f"{bn}{g}", name=f"{bn}{g}_t")
                    nc.sync.dma_start(bt[:], b_d[bn, g][:])
                    b_tiles[bn, g] = bt
            ident = consts.tile([P, P], F32R, tag="ident")
            nc.sync.dma_start(ident[:], id_d[:])
            for j in range(1, NSC):
                for i in range(DC):
                    load_x(i, j)
            load_w("wq", 1)
            load_w("wk", 1)
            load_w("wv", 1)

            # ---- phase 1: QT/KT/VT projections, d-major for weight reuse
            big = {}
            for g in range(NG):
                for tn in ("q", "k", "v"):
                    big[tn, g] = qkv_pool.tile([P, S], F32R, tag=f"{tn}t{g}",
                                               name=f"{tn}t{g}")
            for g in range(NG):
                for j in range(NSC):
                    for tn, wn, bn in (("q", "wq", "bq"), ("k", "wk", "bk"),
                                       ("v", "wv", "bv")):
                        ps = pproj.tile([P, SC], F32, tag="proj",
                                        name=f"ps_{tn}{g}_{j}")
                        for i in range(DC):
                            nc.tensor.matmul(
                                ps[:], w_tiles[wn, g][:, i, :], xs[i, j][:],
                                start=(i == 0), stop=(i == DC - 1),
                            )
                        sl = big[tn, g][:, j * SC:(j + 1) * SC]
                        if tn == "v":
                            nc.scalar.activation(
                                sl, ps[:], mybir.ActivationFunctionType.Identity,
                                bias=b_tiles[bn, g][:])
                        else:
                            nc.vector.tensor_scalar_add(sl, ps[:], b_tiles[bn, g][:])

            # ---- phase 2: M_g = K^T V via PE transposes
            m_tiles = {}
            for g in range(NG):
                mps = pm.tile([P, P], F32, tag="m", name=f"mps_{g}")
                for t in range(TC):
                    sl = slice(t * P, (t + 1) * P)
                    ktp = ptr.tile([P, P], F32R, tag="tr", name=f"ktp_{g}_{t}")
                    nc.tensor.transpose(ktp[:], big["k", g][:, sl], ident[:])
                    k_sb = kv_pool.tile([P, P], F32R, tag="k_sb", name=f"k_sb_{g}_{t}")
                    nc.scalar.copy(k_sb[:], ktp[:])
                    vtp = ptr.tile([P, P], F32R, tag="tr", name=f"vtp_{g}_{t}")
                    nc.tensor.transpose(vtp[:], big["v", g][:, sl], ident[:])
                    v_sb = kv_pool.tile([P, P], F32R, tag="v_sb", name=f"v_sb_{g}_{t}")
                    nc.vector.tensor_copy(v_sb[:], vtp[:])
                    nc.tensor.matmul(mps[:], k_sb[:], v_sb[:],
                                     start=(t == 0), stop=(t == TC - 1))
                mt = mt_pool.tile([P, P], F32R, tag=f"mt{g}", name=f"mt_{g}")
                # zero-fill without InstMemset (walrus rejects f32r memset)
                nc.vector.tensor_scalar_mul(mt[:], ident[:], 0.0)
                nc.vector.tensor_copy(mt[0:E, 0:E], mps[0:E, 0:E])
                nc.vector.tensor_copy(mt[E:P, E:P], mps[E:P, E:P])
                m_tiles[g] = mt

            # ---- phase 3: outT[g] = M_g.T @ QT[g]
            for g in range(NG):
                for j in range(NSC):
                    sl = slice(j * SC, (j + 1) * SC)
                    ps = pproj.tile([P, SC], F32, tag="proj", name=f"ops_{g}_{j}")
                    nc.tensor.matmul(ps[:], m_tiles[g][:], big["q", g][:, sl],
                                     start=True, stop=True)
                    ot = ot_pool.tile([P, SC], F32, tag="ot", name=f"ot_{g}_{j}")
                    nc.vector.tensor_copy(ot[:], ps[:])
                    nc.sync.dma_start(out_d[g, :, sl], ot[:])

    nc.compile()
    return nc


def _prep_inputs(x, Wq, Wk, Wv, bq, bk, bv):
    """Host-side shard + layout prep. Returns per-core input maps."""
    x_tiles_b = []
    for b in range(B):
        xf = np.ascontiguousarray(x[b].T)                   # [D, S]
        x_tiles_b.append(np.ascontiguousarray(
            xf.reshape(DC, P, NSC, SC).transpose(0, 2, 1, 3)))

    def wlayout(w):                                         # [P, D] -> [P, DC, P]
        return np.ascontiguousarray(w.T.reshape(DC, P, P).transpose(1, 0, 2))

    in_maps = []
    for c in range(NCORES):
        b = c // NB
        q0 = HL * (c % NB)                                  # first head of core
        m = {"x": x_tiles_b[b], "ident": np.eye(P, dtype=np.float32)}
        for g in range(NG):
            hs = slice(q0 + 2 * g, q0 + 2 * g + 2)
            m[f"wq{g}"] = wlayout((Wq[hs].reshape(P, D) * NORM).astype(np.float32))
            m[f"wk{g}"] = wlayout(Wk[hs].reshape(P, D).astype(np.float32))
            m[f"wv{g}"] = wlayout(Wv[hs].reshape(P, D).astype(np.float32))
            m[f"bq{g}"] = (bq[hs].reshape(P, 1) * NORM).astype(np.float32)
            m[f"bk{g}"] = bk[hs].reshape(P, 1).astype(np.float32)
            m[f"bv{g}"] = bv[hs].reshape(P, 1).astype(np.float32)
        in_maps.append(m)
    return in_maps


def _gather(results):
    out = np.empty((B, S, D), dtype=np.float32)
    for c in range(NCORES):
        b = c // NB
        oc = results[c]["outT"]                             # [NG, P, S]
        for g in range(NG):
            f0 = (c % NB) * (HL * E) + g * P
            out[b, :, f0:f0 + P] = oc[g].T
    return out


def get_compiled():
    global _compiled
    if _compiled is None:
        _compiled = _build()
    return _compiled


def run(in_maps):
    nc = get_compiled()
    return bass2jax.run_bass_via_pjrt(nc, in_maps, n_cores=NCORES)


def kernel(x, Wq, Wk, Wv, bq, bk, bv):
    in_maps = _prep_inputs(
        np.asarray(x, np.float32), np.asarray(Wq, np.float32),
        np.asarray(Wk, np.float32), np.asarray(Wv, np.float32),
        np.asarray(bq, np.float32), np.asarray(bk, np.float32),
        np.asarray(bv, np.float32),
    )
    return _gather(run(in_maps))


# revision 11
# speedup vs baseline: 1.2627x; 1.0202x over previous
"""Multi-head attention (no softmax) on 8 TRN2 NeuronCores.

Problem: x[2,2048,1024], per-head Wq/Wk/Wv[16,64,1024] + biases.
    q = einsum('bsd,hed->bhse', x, Wq) + bq   (same for k, v)
    out = ((q @ k^T) * E^-0.5) @ v, heads concatenated on feature dim.

Key algebraic fact: there is NO softmax, so
    (q k^T * norm) v = q @ (norm * (k^T v))
which collapses the O(S^2) attention into a 64x64 (per head) matmul.

Sharding: 2D tensor-parallel over (batch, head-quad): core c owns batch
c//4 and heads 4*(c%4) .. 4*(c%4)+3.  Each core reads only its batch's
half of x (8.4MB), and processes its 4 heads as two packed head-pairs
(feature groups g=0,1 of 128).

Per core:
  phase 1: project QT/KT/VT[g] = W[g] @ x_b^T in [feat(128), seq(2048)]
           layout (fp32r matmuls, N=512 moving dim -> full PE speed,
           d-major inner loop so the stationary weights are reused by 4
           consecutive matmuls).  norm is folded into Wq/bq on the host;
           biases are per-partition adds fused into the PSUM->SBUF copies.
  phase 2: per g: PE-transpose K/V tiles to [seq, feat] and accumulate
           M_g = K^T V [128,128] in PSUM over 16 seq-chunks; copy the two
           diagonal 64x64 head blocks into a zeroed SBUF tile (cross-head
           blocks of M are garbage and must be dropped).
  phase 3: outT[g][:, s-chunk] = M_g(blockdiag).T @ QT[g][:, s-chunk].
Host gathers: out[c//4, s, (c%4)*256+g*128 +:128] = outT_c[g][:, s].T
"""

import numpy as np

import concourse.bacc as bacc
import concourse.tile as tile
import concourse.mybir as mybir
from concourse import bass2jax

B, S, D, H = 2, 2048, 1024, 16
E = 64          # head dim
NCORES = 8
NB = NCORES // B            # cores per batch (4)
HL = H // NB                # heads per core (4)
NG = 2                      # feature groups per core (head pairs)
EP = HL * E // NG           # packed feature dim per group (128)
P = 128                     # partitions
DC = D // P                 # d chunks (8)
SC = 512                    # seq chunk for N=512 matmuls
NSC = S // SC               # 4 seq chunks
HW = 1024                   # phase-1 psum half-width (2 banks)
NHW = S // HW               # 2 halves
TC = S // P                 # 16 transpose chunks per group
NORM = float(E) ** -0.5

F32 = mybir.dt.float32
F32R = mybir.dt.float32r

_compiled = None


def _build():
    nc = bacc.Bacc("TRN2", target_bir_lowering=False, debug=False)

    x_d = nc.dram_tensor("x", [DC, NSC, P, SC], F32R, kind="ExternalInput").ap()
    w_d = {}
    for g in range(NG):
        for wn in ("wq", "wk", "wv"):
            w_d[wn, g] = nc.dram_tensor(
                f"{wn}{g}", [P, DC, P], F32R, kind="ExternalInput").ap()
    b_d = {}
    for g in range(NG):
        for bn in ("bq", "bk", "bv"):
            b_d[bn, g] = nc.dram_tensor(
                f"{bn}{g}", [P, 1], F32, kind="ExternalInput").ap()
    id_d = nc.dram_tensor("ident", [P, P], F32R, kind="ExternalInput").ap()
    out_d = nc.dram_tensor("outT", [NG, P, S], F32, kind="ExternalOutput").ap()

    with tile.TileContext(nc) as tc:
        with (
            tc.tile_pool(name="consts", bufs=1) as consts,
            tc.tile_pool(name="xs", bufs=32) as xs_pool,
            tc.tile_pool(name="qkv", bufs=1) as qkv_pool,
            tc.tile_pool(name="kv", bufs=4) as kv_pool,
            tc.tile_pool(name="mt", bufs=1) as mt_pool,
            tc.tile_pool(name="ot", bufs=3) as ot_pool,
            tc.tile_pool(name="pproj", bufs=6, space="PSUM") as pproj,
            tc.tile_pool(name="pm", bufs=2, space="PSUM") as pm,
        ):
            # ---- x stream DMAs for the first half, then weights, then rest.
            # Issue order matters: the DMA queue drains in order, and the
            # first matmul only needs wq0 + x[0,0].
            xs = {}

            def load_x(i, j):
                t = xs_pool.tile([P, SC], F32R, tag="xs", name=f"x_{i}_{j}")
                nc.sync.dma_start(t[:], x_d[i, j])
                xs[i, j] = t

            w_tiles, b_tiles = {}, {}

            def load_w(wn, g, split=False):
                wt = consts.tile([P, DC, P], F32R, tag=f"{wn}{g}", name=f"{wn}{g}_t")
                if split:
                    for i in range(DC):
                        nc.sync.dma_start(wt[:, i, :], w_d[wn, g][:, i, :])
                else:
                    nc.sync.dma_start(wt[:], w_d[wn, g][:])
                w_tiles[wn, g] = wt

            # interleave the first weight chunks with the first x tiles so
            # the first accumulation group can start after ~0.5MB of DMA
            wq0 = consts.tile([P, DC, P], F32R, tag="wq0", name="wq0_t")
            w_tiles["wq", 0] = wq0
            for i in range(DC):
                nc.sync.dma_start(wq0[:, i, :], w_d["wq", 0][:, i, :])
                load_x(i, 0)
            load_w("wk", 0)
            load_w("wv", 0)
            for g in range(NG):
                for bn in ("bq", "bk", "bv"):
                    bt = consts.tile([P, 1], F32, tag=f"{bn}{g}", name=f"{bn}{g}_t")
                    nc.sync.dma_start(bt[:], b_d[bn, g][:])
                    b_tiles[bn, g] = bt
            ident = consts.tile([P, P], F32R, tag="ident")
            nc.sync.dma_start(ident[:], id_d[:])
            for j in range(1, NSC):
                for i in range(DC):
                    load_x(i, j)
            load_w("wq", 1)
            load_w("wk", 1)
            load_w("wv", 1)

            # ---- phase 1: QT/KT/VT projections, d-major for weight reuse
            big = {}
            for g in range(NG):
                for tn in ("q", "k", "v"):
                    big[tn, g] = qkv_pool.tile([P, S], F32R, tag=f"{tn}t{g}",
                                               name=f"{tn}t{g}")
            for g in range(NG):
                for j in range(NSC):
                    for tn, wn, bn in (("q", "wq", "bq"), ("k", "wk", "bk"),
                                       ("v", "wv", "bv")):
                        ps = pproj.tile([P, SC], F32, tag="proj",
                                        name=f"ps_{tn}{g}_{j}")
                        for i in range(DC):
                            nc.tensor.matmul(
                                ps[:], w_tiles[wn, g][:, i, :], xs[i, j][:],
                                start=(i == 0), stop=(i == DC - 1),
                            )
                        sl = big[tn, g][:, j * SC:(j + 1) * SC]
                        if tn == "v":
                            nc.scalar.activation(
                                sl, ps[:], mybir.ActivationFunctionType.Identity,
                                bias=b_tiles[bn, g][:])
                        else:
                            nc.vector.tensor_scalar_add(sl, ps[:], b_tiles[bn, g][:])

            # ---- phase 2: M_g = K^T V via PE transposes.
            # Software-pipelined by one step: the M matmul for chunk t is
            # issued after the transposes for chunk t+1, so the PSUM->SBUF
            # copies of chunk t hide under the t+1 transposes on PE.
            m_tiles = {}
            for g in range(NG):
                mps = pm.tile([P, P], F32, tag="m", name=f"mps_{g}")
                pending = None
                for t in range(TC):
                    sl = slice(t * P, (t + 1) * P)
                    ktp = pproj.tile([P, P], F32R, tag="proj", name=f"ktp_{g}_{t}")
                    nc.tensor.transpose(ktp[:], big["k", g][:, sl], ident[:])
                    k_sb = kv_pool.tile([P, P], F32R, tag="k_sb", name=f"k_sb_{g}_{t}")
                    nc.scalar.copy(k_sb[:], ktp[:])
                    vtp = pproj.tile([P, P], F32R, tag="proj", name=f"vtp_{g}_{t}")
                    nc.tensor.transpose(vtp[:], big["v", g][:, sl], ident[:])
                    v_sb = kv_pool.tile([P, P], F32R, tag="v_sb", name=f"v_sb_{g}_{t}")
                    nc.vector.tensor_copy(v_sb[:], vtp[:])
                    if pending is not None:
                        nc.tensor.matmul(mps[:], pending[0][:], pending[1][:],
                                         start=(pending[2] == 0), stop=False)
                    pending = (k_sb, v_sb, t)
                nc.tensor.matmul(mps[:], pending[0][:], pending[1][:],
                                 start=False, stop=True)
                mt = mt_pool.tile([P, P], F32R, tag=f"mt{g}", name=f"mt_{g}")
                # zero-fill without InstMemset (walrus rejects f32r memset)
                nc.vector.tensor_scalar_mul(mt[:], ident[:], 0.0)
                nc.vector.tensor_copy(mt[0:E, 0:E], mps[0:E, 0:E])
                nc.vector.tensor_copy(mt[E:P, E:P], mps[E:P, E:P])
                m_tiles[g] = mt

            # ---- phase 3: outT[g] = M_g.T @ QT[g]
            for g in range(NG):
                for j in range(NSC):
                    sl = slice(j * SC, (j + 1) * SC)
                    ps = pproj.tile([P, SC], F32, tag="proj", name=f"ops_{g}_{j}")
                    nc.tensor.matmul(ps[:], m_tiles[g][:], big["q", g][:, sl],
                                     start=True, stop=True)
                    ot = ot_pool.tile([P, SC], F32, tag="ot", name=f"ot_{g}_{j}")
                    nc.vector.tensor_copy(ot[:], ps[:])
                    nc.sync.dma_start(out_d[g, :, sl], ot[:])

    nc.compile()
    return nc


def _prep_inputs(x, Wq, Wk, Wv, bq, bk, bv):
    """Host-side shard + layout prep. Returns per-core input maps."""
    x_tiles_b = []
    for b in range(B):
        xf = np.ascontiguousarray(x[b].T)                   # [D, S]
        x_tiles_b.append(np.ascontiguousarray(
            xf.reshape(DC, P, NSC, SC).transpose(0, 2, 1, 3)))

    def wlayout(w):                                         # [P, D] -> [P, DC, P]
        return np.ascontiguousarray(w.T.reshape(DC, P, P).transpose(1, 0, 2))

    in_maps = []
    for c in range(NCORES):
        b = c // NB
        q0 = HL * (c % NB)                                  # first head of core
        m = {"x": x_tiles_b[b], "ident": np.eye(P, dtype=np.float32)}
        for g in range(NG):
            hs = slice(q0 + 2 * g, q0 + 2 * g + 2)
            m[f"wq{g}"] = wlayout((Wq[hs].reshape(P, D) * NORM).astype(np.float32))
            m[f"wk{g}"] = wlayout(Wk[hs].reshape(P, D).astype(np.float32))
            m[f"wv{g}"] = wlayout(Wv[hs].reshape(P, D).astype(np.float32))
            m[f"bq{g}"] = (bq[hs].reshape(P, 1) * NORM).astype(np.float32)
            m[f"bk{g}"] = bk[hs].reshape(P, 1).astype(np.float32)
            m[f"bv{g}"] = bv[hs].reshape(P, 1).astype(np.float32)
        in_maps.append(m)
    return in_maps


def _gather(results):
    out = np.empty((B, S, D), dtype=np.float32)
    for c in range(NCORES):
        b = c // NB
        oc = results[c]["outT"]                             # [NG, P, S]
        for g in range(NG):
            f0 = (c % NB) * (HL * E) + g * P
            out[b, :, f0:f0 + P] = oc[g].T
    return out


def get_compiled():
    global _compiled
    if _compiled is None:
        _compiled = _build()
    return _compiled


def run(in_maps):
    nc = get_compiled()
    return bass2jax.run_bass_via_pjrt(nc, in_maps, n_cores=NCORES)


def kernel(x, Wq, Wk, Wv, bq, bk, bv):
    in_maps = _prep_inputs(
        np.asarray(x, np.float32), np.asarray(Wq, np.float32),
        np.asarray(Wk, np.float32), np.asarray(Wv, np.float32),
        np.asarray(bq, np.float32), np.asarray(bk, np.float32),
        np.asarray(bv, np.float32),
    )
    return _gather(run(in_maps))


# revision 14
# speedup vs baseline: 1.3156x; 1.0419x over previous
"""Multi-head attention (no softmax) on 8 TRN2 NeuronCores.

Problem: x[2,2048,1024], per-head Wq/Wk/Wv[16,64,1024] + biases.
    q = einsum('bsd,hed->bhse', x, Wq) + bq   (same for k, v)
    out = ((q @ k^T) * E^-0.5) @ v, heads concatenated on feature dim.

Key algebraic fact: there is NO softmax, so
    (q k^T * norm) v = q @ (norm * (k^T v))
which collapses the O(S^2) attention into a 64x64 (per head) matmul.

Sharding: 2D tensor-parallel over (batch, head-quad): core c owns batch
c//4 and heads 4*(c%4) .. 4*(c%4)+3.  Each core reads only its batch's
half of x (8.4MB), and processes its 4 heads as two packed head-pairs
(feature groups g=0,1 of 128).

Per core:
  phase 1: project QT/KT/VT[g] = W[g] @ x_b^T in [feat(128), seq(2048)]
           layout (fp32r matmuls, N=512 moving dim -> full PE speed,
           d-major inner loop so the stationary weights are reused by 4
           consecutive matmuls).  norm is folded into Wq/bq on the host;
           biases are per-partition adds fused into the PSUM->SBUF copies.
  phase 2: per g: PE-transpose K/V tiles to [seq, feat] and accumulate
           M_g = K^T V [128,128] in PSUM over 16 seq-chunks; copy the two
           diagonal 64x64 head blocks into a zeroed SBUF tile (cross-head
           blocks of M are garbage and must be dropped).
  phase 3: outT[g][:, s-chunk] = M_g(blockdiag).T @ QT[g][:, s-chunk].
Host gathers: out[c//4, s, (c%4)*256+g*128 +:128] = outT_c[g][:, s].T
"""

import numpy as np

import concourse.bacc as bacc
import concourse.tile as tile
import concourse.mybir as mybir
from concourse import bass2jax

B, S, D, H = 2, 2048, 1024, 16
E = 64          # head dim
NCORES = 8
NB = NCORES // B            # cores per batch (4)
HL = H // NB                # heads per core (4)
NG = 2                      # feature groups per core (head pairs)
EP = HL * E // NG           # packed feature dim per group (128)
P = 128                     # partitions
DC = D // P                 # d chunks (8)
SC = 512                    # seq chunk for N=512 matmuls
NSC = S // SC               # 4 seq chunks
HW = 1024                   # phase-1 psum half-width (2 banks)
NHW = S // HW               # 2 halves
TC = S // P                 # 16 transpose chunks per group
NORM = float(E) ** -0.5

F32 = mybir.dt.float32
F32R = mybir.dt.float32r

_compiled = None


def _build():
    nc = bacc.Bacc("TRN2", target_bir_lowering=False, debug=False)

    x_d = nc.dram_tensor("x", [DC, NSC, P, SC], F32R, kind="ExternalInput").ap()
    w_d = {}
    for g in range(NG):
        for wn in ("wq", "wk", "wv"):
            w_d[wn, g] = nc.dram_tensor(
                f"{wn}{g}", [P, DC, P], F32R, kind="ExternalInput").ap()
    b_d = {}
    for g in range(NG):
        for bn in ("bq", "bk", "bv"):
            b_d[bn, g] = nc.dram_tensor(
                f"{bn}{g}", [P, 1], F32, kind="ExternalInput").ap()
    id_d = nc.dram_tensor("ident", [P, P], F32R, kind="ExternalInput").ap()
    out_d = nc.dram_tensor("outT", [NG, P, S], F32, kind="ExternalOutput").ap()

    with tile.TileContext(nc) as tc:
        with (
            tc.tile_pool(name="consts", bufs=1) as consts,
            tc.tile_pool(name="xs", bufs=32) as xs_pool,
            tc.tile_pool(name="qkv", bufs=1) as qkv_pool,
            tc.tile_pool(name="kv", bufs=4) as kv_pool,
            tc.tile_pool(name="mt", bufs=1) as mt_pool,
            tc.tile_pool(name="ot", bufs=8) as ot_pool,
            tc.tile_pool(name="pproj", bufs=6, space="PSUM") as pproj,
            tc.tile_pool(name="pm", bufs=2, space="PSUM") as pm,
        ):
            # ---- x stream DMAs for the first half, then weights, then rest.
            # Issue order matters: the DMA queue drains in order, and the
            # first matmul only needs wq0 + x[0,0].
            xs = {}

            def load_x(i, j):
                t = xs_pool.tile([P, SC], F32R, tag="xs", name=f"x_{i}_{j}")
                nc.sync.dma_start(t[:], x_d[i, j])
                xs[i, j] = t

            w_tiles, b_tiles = {}, {}

            def load_w(wn, g, split=False):
                wt = consts.tile([P, DC, P], F32R, tag=f"{wn}{g}", name=f"{wn}{g}_t")
                if split:
                    for i in range(DC):
                        nc.sync.dma_start(wt[:, i, :], w_d[wn, g][:, i, :])
                else:
                    nc.sync.dma_start(wt[:], w_d[wn, g][:])
                w_tiles[wn, g] = wt

            # interleave the first weight chunks with the first x tiles so
            # the first accumulation group can start after ~0.5MB of DMA
            wq0 = consts.tile([P, DC, P], F32R, tag="wq0", name="wq0_t")
            w_tiles["wq", 0] = wq0
            for i in range(DC):
                nc.sync.dma_start(wq0[:, i, :], w_d["wq", 0][:, i, :])
                load_x(i, 0)
            load_w("wk", 0)
            load_w("wv", 0)
            for g in range(NG):
                for bn in ("bq", "bk", "bv"):
                    bt = consts.tile([P, 1], F32, tag=f"{bn}{g}", name=f"{bn}{g}_t")
                    nc.sync.dma_start(bt[:], b_d[bn, g][:])
                    b_tiles[bn, g] = bt
            ident = consts.tile([P, P], F32R, tag="ident")
            nc.sync.dma_start(ident[:], id_d[:])
            for j in range(1, NSC):
                for i in range(DC):
                    load_x(i, j)
            load_w("wq", 1)
            load_w("wk", 1)
            load_w("wv", 1)

            # ---- phase 1: QT/KT/VT projections, d-major for weight reuse
            big = {}
            for g in range(NG):
                for tn in ("q", "k", "v"):
                    big[tn, g] = qkv_pool.tile([P, S], F32R, tag=f"{tn}t{g}",
                                               name=f"{tn}t{g}")
            for g in range(NG):
                for j in range(NSC):
                    for tn, wn, bn in (("q", "wq", "bq"), ("k", "wk", "bk"),
                                       ("v", "wv", "bv")):
                        ps = pproj.tile([P, SC], F32, tag="proj",
                                        name=f"ps_{tn}{g}_{j}")
                        for i in range(DC):
                            nc.tensor.matmul(
                                ps[:], w_tiles[wn, g][:, i, :], xs[i, j][:],
                                start=(i == 0), stop=(i == DC - 1),
                            )
                        sl = big[tn, g][:, j * SC:(j + 1) * SC]
                        if tn == "v":
                            nc.scalar.activation(
                                sl, ps[:], mybir.ActivationFunctionType.Identity,
                                bias=b_tiles[bn, g][:])
                        else:
                            nc.vector.tensor_scalar_add(sl, ps[:], b_tiles[bn, g][:])

            # ---- phases 2+3 per group, interleaved so group 0's output
            # matmuls/copies/DMAs overlap group 1's transposes.
            def phase2(g):
                mps = pm.tile([P, P], F32, tag="m", name=f"mps_{g}")
                pending = None
                for t in range(TC):
                    sl = slice(t * P, (t + 1) * P)
                    ktp = pproj.tile([P, P], F32R, tag="proj", name=f"ktp_{g}_{t}")
                    nc.tensor.transpose(ktp[:], big["k", g][:, sl], ident[:])
                    k_sb = kv_pool.tile([P, P], F32R, tag="k_sb", name=f"k_sb_{g}_{t}")
                    nc.scalar.copy(k_sb[:], ktp[:])
                    vtp = pproj.tile([P, P], F32R, tag="proj", name=f"vtp_{g}_{t}")
                    nc.tensor.transpose(vtp[:], big["v", g][:, sl], ident[:])
                    v_sb = kv_pool.tile([P, P], F32R, tag="v_sb", name=f"v_sb_{g}_{t}")
                    nc.vector.tensor_copy(v_sb[:], vtp[:])
                    if pending is not None:
                        nc.tensor.matmul(mps[:], pending[0][:], pending[1][:],
                                         start=(pending[2] == 0), stop=False)
                    pending = (k_sb, v_sb, t)
                nc.tensor.matmul(mps[:], pending[0][:], pending[1][:],
                                 start=False, stop=True)
                mt = mt_pool.tile([P, P], F32R, tag=f"mt{g}", name=f"mt_{g}")
                # zero-fill without InstMemset (walrus rejects f32r memset)
                nc.vector.tensor_scalar_mul(mt[:], ident[:], 0.0)
                nc.vector.tensor_copy(mt[0:E, 0:E], mps[0:E, 0:E])
                nc.vector.tensor_copy(mt[E:P, E:P], mps[E:P, E:P])
                return mt

            def phase3(g, mt):
                for j in range(NSC):
                    sl = slice(j * SC, (j + 1) * SC)
                    ps = pproj.tile([P, SC], F32, tag="proj", name=f"ops_{g}_{j}")
                    nc.tensor.matmul(ps[:], mt[:], big["q", g][:, sl],
                                     start=True, stop=True)
                    ot = ot_pool.tile([P, SC], F32, tag="ot", name=f"ot_{g}_{j}")
                    nc.vector.tensor_copy(ot[:], ps[:])
                    nc.sync.dma_start(out_d[g, :, sl], ot[:])

            for g in range(NG):
                phase3(g, phase2(g))

    nc.compile()
    return nc


def _prep_inputs(x, Wq, Wk, Wv, bq, bk, bv):
    """Host-side shard + layout prep. Returns per-core input maps."""
    x_tiles_b = []
    for b in range(B):
        xf = np.ascontiguousarray(x[b].T)                   # [D, S]
        x_tiles_b.append(np.ascontiguousarray(
            xf.reshape(DC, P, NSC, SC).transpose(0, 2, 1, 3)))

    def wlayout(w):                                         # [P, D] -> [P, DC, P]
        return np.ascontiguousarray(w.T.reshape(DC, P, P).transpose(1, 0, 2))

    in_maps = []
    for c in range(NCORES):
        b = c // NB
        q0 = HL * (c % NB)                                  # first head of core
        m = {"x": x_tiles_b[b], "ident": np.eye(P, dtype=np.float32)}
        for g in range(NG):
            hs = slice(q0 + 2 * g, q0 + 2 * g + 2)
            m[f"wq{g}"] = wlayout((Wq[hs].reshape(P, D) * NORM).astype(np.float32))
            m[f"wk{g}"] = wlayout(Wk[hs].reshape(P, D).astype(np.float32))
            m[f"wv{g}"] = wlayout(Wv[hs].reshape(P, D).astype(np.float32))
            m[f"bq{g}"] = (bq[hs].reshape(P, 1) * NORM).astype(np.float32)
            m[f"bk{g}"] = bk[hs].reshape(P, 1).astype(np.float32)
            m[f"bv{g}"] = bv[hs].reshape(P, 1).astype(np.float32)
        in_maps.append(m)
    return in_maps


def _gather(results):
    out = np.empty((B, S, D), dtype=np.float32)
    for c in range(NCORES):
        b = c // NB
        oc = results[c]["outT"]                             # [NG, P, S]
        for g in range(NG):
            f0 = (c % NB) * (HL * E) + g * P
            out[b, :, f0:f0 + P] = oc[g].T
    return out


def get_compiled():
    global _compiled
    if _compiled is None:
        _compiled = _build()
    return _compiled


def run(in_maps):
    nc = get_compiled()
    return bass2jax.run_bass_via_pjrt(nc, in_maps, n_cores=NCORES)


def kernel(x, Wq, Wk, Wv, bq, bk, bv):
    in_maps = _prep_inputs(
        np.asarray(x, np.float32), np.asarray(Wq, np.float32),
        np.asarray(Wk, np.float32), np.asarray(Wv, np.float32),
        np.asarray(bq, np.float32), np.asarray(bk, np.float32),
        np.asarray(bv, np.float32),
    )
    return _gather(run(in_maps))
